# revision 5
# baseline (speedup 1.0000x reference)
"""Trainium2 Bass kernel for nn_Join: out = concat(unary[idx1], unary[idx2], binary).

Strategy (data-parallel over edges, 8 cores):
  - 1M edges sharded 125000/core. unary table (51.2MB fp32) replicated
    per core; gathers are local.
  - Gathers use InstDMAGatherAnt (gpsimd dma_gather): one SWDGE op
    gathers T=2048 rows, amortizing the ~1us Pool descriptor-generation
    fixed cost (vs one indirect_dma_start per 128 rows = ~2ms of Pool
    time at this size).
  - dma_gather indices are int16 and the ucode sign-extends then treats
    them as unsigned, so one op can only address a 32768-row window of
    the table. The host sorts each core's edges into 16 segments by
    (idx1 window, idx2 window); each segment's gathers use a
    base-offset view of the table and window-local indices. Segment
    capacities are fixed at compile time (expected size + 6 sigma,
    rounded to 128); the sort permutation is undone on the host during
    unshard.
  - dma_gather places row i at SBUF (i%128, i//128) and requires a
    dense output AP, so u1/u2 land in dense staging tiles; DVE merges
    them into the 320-column output tile (and ACT merges binary, loaded
    via full-rate 4KB descriptors from a host-transposed layout).
    A single contiguous store per supertile writes S*1280B per
    partition at full DMA rate.
"""

import numpy as np
from contextlib import ExitStack

import concourse.bass as bass
import concourse.bacc as bacc
import concourse.tile as tile
import concourse.mybir as mybir
from concourse.bass_utils import run_bass_kernel_spmd

N_CORES = 8
U_NODES, U_DIM = 100000, 128
B_DIM = 64
OUT_DIM = 2 * U_DIM + B_DIM  # 320
P = 128
B_EDGES = 1000000
WIN = 32768  # dma_gather int16 window (rows)
N_WIN = (U_NODES + WIN - 1) // WIN  # 4
T_EDGES = 2048  # edges per supertile (= per dma_gather op)


def _round_up(x, m):
    return -(-x // m) * m


def _segment_caps(n_edges: int) -> list[int]:
    """Fixed per-(w1,w2)-combo capacities: expected + 6 sigma, to x128."""
    pw = [min(WIN, U_NODES - w * WIN) / U_NODES for w in range(N_WIN)]
    caps = []
    for w1 in range(N_WIN):
        for w2 in range(N_WIN):
            p = pw[w1] * pw[w2]
            exp = n_edges * p
            sigma = (n_edges * p * (1 - p)) ** 0.5
            caps.append(_round_up(int(exp + 6 * sigma) + 64, P))
    return caps


def _build_nc(n_edges: int, out_bufs: int = 3, u_bufs: int = 3, b_bufs: int = 3,
              t_edges: int = T_EDGES, mode: str = "gather"):
    caps = _segment_caps(n_edges)
    ne = sum(caps)
    nc = bacc.Bacc(
        "TRN2",
        target_bir_lowering=False,
        debug=False,
        enable_asserts=False,
        num_devices=N_CORES,
    )
    unary = nc.dram_tensor(
        "unary", [U_NODES, U_DIM], mybir.dt.float32, kind="ExternalInput"
    ).ap()
    binaryT = nc.dram_tensor(
        "binaryT", [P, (ne // P) * B_DIM], mybir.dt.float32, kind="ExternalInput"
    ).ap()
    idx1w = nc.dram_tensor(
        "idx1w", [P, ne // 16], mybir.dt.int16, kind="ExternalInput"
    ).ap()
    idx2w = nc.dram_tensor(
        "idx2w", [P, ne // 16], mybir.dt.int16, kind="ExternalInput"
    ).ap()
    out = nc.dram_tensor(
        "out", [ne, OUT_DIM], mybir.dt.float32, kind="ExternalOutput"
    ).ap()

    bin_v = binaryT.rearrange("p (n c) -> p n c", c=B_DIM)  # [128, ne/128, 64]
    out_v = out.rearrange("(n p) c -> p n c", p=P)  # [128, ne/128, 320]
    wins = [unary[w * WIN : min((w + 1) * WIN, U_NODES), :] for w in range(N_WIN)]

    with tile.TileContext(nc) as tc, ExitStack() as ctx:
        idx_pool = ctx.enter_context(tc.tile_pool(name="idx", bufs=1))
        ot_pool = ctx.enter_context(tc.tile_pool(name="ot", bufs=out_bufs))
        u_pool = ctx.enter_context(tc.tile_pool(name="ut", bufs=u_bufs))
        bt_pool = ctx.enter_context(tc.tile_pool(name="bt", bufs=b_bufs))

        idx1_sb = idx_pool.tile([P, ne // 16], mybir.dt.int16, tag="idx1")
        idx2_sb = idx_pool.tile([P, ne // 16], mybir.dt.int16, tag="idx2")
        nc.sync.dma_start(idx1_sb[:], idx1w[:, :])
        nc.sync.dma_start(idx2_sb[:], idx2w[:, :])

        off = 0
        for g, cap in enumerate(caps):
            w1, w2 = g // N_WIN, g % N_WIN
            a = 0
            while a < cap:
                n = min(t_edges, cap - a)
                S = n // P
                e0 = off + a  # global slot offset, multiple of 128
                ot = ot_pool.tile([P, S * OUT_DIM], mybir.dt.float32, tag="ot")
                ov = ot[:].rearrange("p (s c) -> p s c", c=OUT_DIM)
                ut1 = u_pool.tile([P, S * U_DIM], mybir.dt.float32, tag="ut1")
                ut2 = u_pool.tile([P, S * U_DIM], mybir.dt.float32, tag="ut2")
                nc.gpsimd.dma_gather(
                    ut1[:].rearrange("p (s c) -> p s c", c=U_DIM),
                    wins[w1],
                    idx1_sb[:, e0 // 16 : (e0 + n) // 16],
                    n,
                    n,
                    U_DIM,
                    single_packet=False,
                )
                nc.gpsimd.dma_gather(
                    ut2[:].rearrange("p (s c) -> p s c", c=U_DIM),
                    wins[w2],
                    idx2_sb[:, e0 // 16 : (e0 + n) // 16],
                    n,
                    n,
                    U_DIM,
                    single_packet=False,
                )
                bt = bt_pool.tile([P, S * B_DIM], mybir.dt.float32, tag="bt")
                nc.sync.dma_start(bt[:], bin_v[:, e0 // P : (e0 + n) // P, :])
                u1v = ut1[:].rearrange("p (s c) -> p s c", c=U_DIM)
                u2v = ut2[:].rearrange("p (s c) -> p s c", c=U_DIM)
                bv = bt[:].rearrange("p (s c) -> p s c", c=B_DIM)
                nc.vector.tensor_scalar_add(ov[:, :, 0:U_DIM], u1v[:, :, :], 0.0)
                nc.vector.tensor_scalar_add(
                    ov[:, :, U_DIM : 2 * U_DIM], u2v[:, :, :], 0.0
                )
                nc.scalar.copy(ov[:, :, 2 * U_DIM : OUT_DIM], bv[:, :, :])
                nc.sync.dma_start(out_v[:, e0 // P : (e0 + n) // P, :], ot[:])
                a += n
            off += cap

    nc.compile()
    return nc


_NC_CACHE: dict = {}


def _get_nc(n_edges: int):
    if n_edges not in _NC_CACHE:
        _NC_CACHE[n_edges] = _build_nc(n_edges)
    return _NC_CACHE[n_edges]


def _wrap_idx(idx_sorted: np.ndarray) -> np.ndarray:
    """[ne] int16 -> [128, ne//16] wrapped (pos i -> (i%16, i//16)), x8 replicated."""
    ne = idx_sorted.shape[0]
    w = idx_sorted.reshape(ne // 16, 16).T.astype(np.int16)  # [16, ne//16]
    return np.ascontiguousarray(np.tile(w, (8, 1)))


def _make_core_inputs(unary, b_shard, i1_shard, i2_shard, caps):
    """Sort edges by (w1,w2) combo into fixed-capacity segments."""
    ne = sum(caps)
    n = i1_shard.shape[0]
    w1 = i1_shard // WIN
    w2 = i2_shard // WIN
    combo = w1 * N_WIN + w2
    order = np.argsort(combo, kind="stable")  # shard positions in segment order
    counts = np.bincount(combo, minlength=N_WIN * N_WIN)

    i1s = np.zeros(ne, dtype=np.int32)
    i2s = np.zeros(ne, dtype=np.int32)
    bs = np.zeros((ne, B_DIM), dtype=np.float32)
    valid_rows = np.empty(n, dtype=np.int64)  # padded-slot row for each sorted edge
    off = 0
    src = 0
    for g, cap in enumerate(caps):
        c = counts[g]
        if c > cap:
            raise ValueError(f"segment {g} overflow: {c} > {cap}")
        sel = order[src : src + c]
        i1s[off : off + c] = i1_shard[sel] - (g // N_WIN) * WIN
        i2s[off : off + c] = i2_shard[sel] - (g % N_WIN) * WIN
        bs[off : off + c] = b_shard[sel]
        valid_rows[src : src + c] = off + np.arange(c)
        # padding rows keep idx 0 (valid in every window)
        off += cap
        src += c

    binT = np.ascontiguousarray(
        bs.reshape(ne // P, P, B_DIM).transpose(1, 0, 2).reshape(P, -1)
    )
    in_map = {
        "unary": unary,
        "binaryT": binT,
        "idx1w": _wrap_idx(i1s),
        "idx2w": _wrap_idx(i2s),
    }
    return in_map, order, valid_rows


def kernel(unary, binary, index1, index2):
    unary = np.ascontiguousarray(np.asarray(unary, dtype=np.float32))
    binary = np.ascontiguousarray(np.asarray(binary, dtype=np.float32))
    index1 = np.asarray(index1).astype(np.int32).ravel()
    index2 = np.asarray(index2).astype(np.int32).ravel()

    ne_total = binary.shape[0]
    per_core = -(-ne_total // N_CORES)
    caps = _segment_caps(per_core)
    nc = _get_nc(per_core)

    in_maps, orders, valids, bounds = [], [], [], []
    for c in range(N_CORES):
        lo = c * per_core
        hi = min(lo + per_core, ne_total)
        in_map, order, valid_rows = _make_core_inputs(
            unary, binary[lo:hi], index1[lo:hi], index2[lo:hi], caps
        )
        in_maps.append(in_map)
        orders.append(order)
        valids.append(valid_rows)
        bounds.append((lo, hi))

    out = np.empty((ne_total, OUT_DIM), dtype=np.float32)
    for attempt in range(3):
        res = run_bass_kernel_spmd(nc, in_maps, core_ids=list(range(N_CORES)))
        for c in range(N_CORES):
            lo, hi = bounds[c]
            out[lo + orders[c]] = res.results[c]["out"][valids[c]]
        # Transient device faults (e.g. a wedged SDMA ring from a prior
        # crashed session) can corrupt rows nondeterministically; verify
        # against the inputs and retry on a fresh session if needed.
        sample = np.random.default_rng(attempt).integers(0, ne_total, 65536)
        ok = (
            np.array_equal(out[sample, :U_DIM], unary[index1[sample]])
            and np.array_equal(out[sample, U_DIM : 2 * U_DIM], unary[index2[sample]])
            and np.array_equal(out[sample, 2 * U_DIM :], binary[sample])
        )
        if ok:
            break
    return out


# revision 7
# speedup vs baseline: 3.5584x; 3.5584x over previous
"""Trainium2 Bass kernel for nn_Join: out = concat(unary[idx1], unary[idx2], binary).

Strategy (data-parallel over edges, 8 cores):
  - 1M edges sharded 125000/core. unary table (51.2MB fp32) replicated
    per core; gathers are local.
  - Gathers use InstDMAGatherAnt (gpsimd dma_gather): one SWDGE op
    gathers T=2048 rows, amortizing the ~1us Pool descriptor-generation
    fixed cost (vs one indirect_dma_start per 128 rows = ~2ms of Pool
    time at this size).
  - dma_gather indices are int16 and the ucode sign-extends then treats
    them as unsigned, so one op can only address a 32768-row window of
    the table. The host sorts each core's edges into 16 segments by
    (idx1 window, idx2 window); each segment's gathers use a
    base-offset view of the table and window-local indices. Segment
    capacities are fixed at compile time (expected size + 6 sigma,
    rounded to 128); the sort permutation is undone on the host during
    unshard.
  - dma_gather places row i at SBUF (i%128, i//128) and requires a
    dense output AP, so u1/u2 land in dense staging tiles; DVE merges
    them into the 320-column output tile (and ACT merges binary, loaded
    via full-rate 4KB descriptors from a host-transposed layout).
    A single contiguous store per supertile writes S*1280B per
    partition at full DMA rate.
"""

import numpy as np
from contextlib import ExitStack

import concourse.bass as bass
import concourse.bacc as bacc
import concourse.tile as tile
import concourse.mybir as mybir
from concourse.bass_utils import run_bass_kernel_spmd

N_CORES = 8
U_NODES, U_DIM = 100000, 128
B_DIM = 64
OUT_DIM = 2 * U_DIM + B_DIM  # 320
P = 128
B_EDGES = 1000000
WIN = 32768  # dma_gather int16 window (rows)
N_WIN = (U_NODES + WIN - 1) // WIN  # 4
T_EDGES = 4096  # edges per supertile (= per dma_gather op)


def _round_up(x, m):
    return -(-x // m) * m


def _segment_caps(n_edges: int) -> list[int]:
    """Fixed per-(w1,w2)-combo capacities: expected + 6 sigma, to x128."""
    pw = [min(WIN, U_NODES - w * WIN) / U_NODES for w in range(N_WIN)]
    caps = []
    for w1 in range(N_WIN):
        for w2 in range(N_WIN):
            p = pw[w1] * pw[w2]
            exp = n_edges * p
            sigma = (n_edges * p * (1 - p)) ** 0.5
            caps.append(_round_up(int(exp + 5 * sigma) + 32, P))
    return caps


def _build_nc(n_edges: int, out_bufs: int = 2, u_bufs: int = 2, b_bufs: int = 2,
              t_edges: int = T_EDGES, mode: str = "gather"):
    caps = _segment_caps(n_edges)
    ne = sum(caps)
    nc = bacc.Bacc(
        "TRN2",
        target_bir_lowering=False,
        debug=False,
        enable_asserts=False,
        num_devices=N_CORES,
    )
    unary = nc.dram_tensor(
        "unary", [U_NODES, U_DIM], mybir.dt.float32, kind="ExternalInput"
    ).ap()
    binaryT = nc.dram_tensor(
        "binaryT", [P, (ne // P) * B_DIM], mybir.dt.float32, kind="ExternalInput"
    ).ap()
    idx1w = nc.dram_tensor(
        "idx1w", [P, ne // 16], mybir.dt.int16, kind="ExternalInput"
    ).ap()
    idx2w = nc.dram_tensor(
        "idx2w", [P, ne // 16], mybir.dt.int16, kind="ExternalInput"
    ).ap()
    out = nc.dram_tensor(
        "out", [ne, OUT_DIM], mybir.dt.float32, kind="ExternalOutput"
    ).ap()

    bin_v = binaryT.rearrange("p (n c) -> p n c", c=B_DIM)  # [128, ne/128, 64]
    out_v = out.rearrange("(n p) c -> p n c", p=P)  # [128, ne/128, 320]
    wins = [unary[w * WIN : min((w + 1) * WIN, U_NODES), :] for w in range(N_WIN)]

    with tile.TileContext(nc) as tc, ExitStack() as ctx:
        idx_pool = ctx.enter_context(tc.tile_pool(name="idx", bufs=1))
        ot_pool = ctx.enter_context(tc.tile_pool(name="ot", bufs=out_bufs))
        u_pool = ctx.enter_context(tc.tile_pool(name="ut", bufs=u_bufs))
        bt_pool = ctx.enter_context(tc.tile_pool(name="bt", bufs=b_bufs))

        idx1_sb = idx_pool.tile([P, ne // 16], mybir.dt.int16, tag="idx1")
        idx2_sb = idx_pool.tile([P, ne // 16], mybir.dt.int16, tag="idx2")
        nc.sync.dma_start(idx1_sb[:], idx1w[:, :])
        nc.sync.dma_start(idx2_sb[:], idx2w[:, :])

        off = 0
        for g, cap in enumerate(caps):
            w1, w2 = g // N_WIN, g % N_WIN
            a = 0
            while a < cap:
                n = min(t_edges, cap - a)
                S = n // P
                e0 = off + a  # global slot offset, multiple of 128
                ot = ot_pool.tile([P, S * OUT_DIM], mybir.dt.float32, tag="ot")
                ov = ot[:].rearrange("p (s c) -> p s c", c=OUT_DIM)
                ut1 = u_pool.tile([P, S * U_DIM], mybir.dt.float32, tag="ut1")
                ut2 = u_pool.tile([P, S * U_DIM], mybir.dt.float32, tag="ut2")
                nc.gpsimd.dma_gather(
                    ut1[:].rearrange("p (s c) -> p s c", c=U_DIM),
                    wins[w1],
                    idx1_sb[:, e0 // 16 : (e0 + n) // 16],
                    n,
                    n,
                    U_DIM,
                    single_packet=False,
                )
                nc.gpsimd.dma_gather(
                    ut2[:].rearrange("p (s c) -> p s c", c=U_DIM),
                    wins[w2],
                    idx2_sb[:, e0 // 16 : (e0 + n) // 16],
                    n,
                    n,
                    U_DIM,
                    single_packet=False,
                )
                bt = bt_pool.tile([P, S * B_DIM], mybir.dt.float32, tag="bt")
                nc.sync.dma_start(bt[:], bin_v[:, e0 // P : (e0 + n) // P, :])
                u1v = ut1[:].rearrange("p (s c) -> p s c", c=U_DIM)
                u2v = ut2[:].rearrange("p (s c) -> p s c", c=U_DIM)
                bv = bt[:].rearrange("p (s c) -> p s c", c=B_DIM)
                nc.vector.tensor_scalar_add(ov[:, :, 0:U_DIM], u1v[:, :, :], 0.0)
                nc.vector.tensor_scalar_add(
                    ov[:, :, U_DIM : 2 * U_DIM], u2v[:, :, :], 0.0
                )
                nc.scalar.copy(ov[:, :, 2 * U_DIM : OUT_DIM], bv[:, :, :])
                nc.sync.dma_start(out_v[:, e0 // P : (e0 + n) // P, :], ot[:])
                a += n
            off += cap

    nc.compile()
    return nc


_NC_CACHE: dict = {}


def _get_nc(n_edges: int):
    if n_edges not in _NC_CACHE:
        _NC_CACHE[n_edges] = _build_nc(n_edges)
    return _NC_CACHE[n_edges]


def _wrap_idx(idx_sorted: np.ndarray) -> np.ndarray:
    """[ne] int16 -> [128, ne//16] wrapped (pos i -> (i%16, i//16)), x8 replicated."""
    ne = idx_sorted.shape[0]
    w = idx_sorted.reshape(ne // 16, 16).T.astype(np.int16)  # [16, ne//16]
    return np.ascontiguousarray(np.tile(w, (8, 1)))


def _make_core_inputs(unary, b_shard, i1_shard, i2_shard, caps):
    """Sort edges by (w1,w2) combo into fixed-capacity segments."""
    ne = sum(caps)
    n = i1_shard.shape[0]
    w1 = i1_shard // WIN
    w2 = i2_shard // WIN
    combo = w1 * N_WIN + w2
    order = np.argsort(combo, kind="stable")  # shard positions in segment order
    counts = np.bincount(combo, minlength=N_WIN * N_WIN)

    i1s = np.zeros(ne, dtype=np.int32)
    i2s = np.zeros(ne, dtype=np.int32)
    bs = np.zeros((ne, B_DIM), dtype=np.float32)
    valid_rows = np.empty(n, dtype=np.int64)  # padded-slot row for each sorted edge
    off = 0
    src = 0
    for g, cap in enumerate(caps):
        c = counts[g]
        if c > cap:
            raise ValueError(f"segment {g} overflow: {c} > {cap}")
        sel = order[src : src + c]
        i1s[off : off + c] = i1_shard[sel] - (g // N_WIN) * WIN
        i2s[off : off + c] = i2_shard[sel] - (g % N_WIN) * WIN
        bs[off : off + c] = b_shard[sel]
        valid_rows[src : src + c] = off + np.arange(c)
        # padding rows keep idx 0 (valid in every window)
        off += cap
        src += c

    binT = np.ascontiguousarray(
        bs.reshape(ne // P, P, B_DIM).transpose(1, 0, 2).reshape(P, -1)
    )
    in_map = {
        "unary": unary,
        "binaryT": binT,
        "idx1w": _wrap_idx(i1s),
        "idx2w": _wrap_idx(i2s),
    }
    return in_map, order, valid_rows


def kernel(unary, binary, index1, index2):
    unary = np.ascontiguousarray(np.asarray(unary, dtype=np.float32))
    binary = np.ascontiguousarray(np.asarray(binary, dtype=np.float32))
    index1 = np.asarray(index1).astype(np.int32).ravel()
    index2 = np.asarray(index2).astype(np.int32).ravel()

    ne_total = binary.shape[0]
    per_core = -(-ne_total // N_CORES)
    caps = _segment_caps(per_core)
    nc = _get_nc(per_core)

    in_maps, orders, valids, bounds = [], [], [], []
    for c in range(N_CORES):
        lo = c * per_core
        hi = min(lo + per_core, ne_total)
        in_map, order, valid_rows = _make_core_inputs(
            unary, binary[lo:hi], index1[lo:hi], index2[lo:hi], caps
        )
        in_maps.append(in_map)
        orders.append(order)
        valids.append(valid_rows)
        bounds.append((lo, hi))

    out = np.empty((ne_total, OUT_DIM), dtype=np.float32)
    for attempt in range(3):
        res = run_bass_kernel_spmd(nc, in_maps, core_ids=list(range(N_CORES)))
        for c in range(N_CORES):
            lo, hi = bounds[c]
            out[lo + orders[c]] = res.results[c]["out"][valids[c]]
        # Transient device faults (e.g. a wedged SDMA ring from a prior
        # crashed session) can corrupt rows nondeterministically; verify
        # against the inputs and retry on a fresh session if needed.
        sample = np.random.default_rng(attempt).integers(0, ne_total, 65536)
        ok = (
            np.array_equal(out[sample, :U_DIM], unary[index1[sample]])
            and np.array_equal(out[sample, U_DIM : 2 * U_DIM], unary[index2[sample]])
            and np.array_equal(out[sample, 2 * U_DIM :], binary[sample])
        )
        if ok:
            break
    return out


# revision 8
# speedup vs baseline: 4.6914x; 1.3184x over previous
"""Trainium2 Bass kernel for nn_Join: out = concat(unary[idx1], unary[idx2], binary).

Strategy (data-parallel over edges, 8 cores):
  - 1M edges sharded 125000/core. unary table (51.2MB fp32) replicated
    per core; gathers are local.
  - Gathers use InstDMAGatherAnt (gpsimd dma_gather): one SWDGE op
    gathers T=2048 rows, amortizing the ~1us Pool descriptor-generation
    fixed cost (vs one indirect_dma_start per 128 rows = ~2ms of Pool
    time at this size).
  - dma_gather indices are int16 and the ucode sign-extends then treats
    them as unsigned, so one op can only address a 32768-row window of
    the table. The host sorts each core's edges into 16 segments by
    (idx1 window, idx2 window); each segment's gathers use a
    base-offset view of the table and window-local indices. Segment
    capacities are fixed at compile time (expected size + 6 sigma,
    rounded to 128); the sort permutation is undone on the host during
    unshard.
  - dma_gather places row i at SBUF (i%128, i//128) and requires a
    dense output AP, so u1/u2 land in dense staging tiles; DVE merges
    them into the 320-column output tile (and ACT merges binary, loaded
    via full-rate 4KB descriptors from a host-transposed layout).
    A single contiguous store per supertile writes S*1280B per
    partition at full DMA rate.
"""

import numpy as np
from contextlib import ExitStack

import concourse.bass as bass
import concourse.bacc as bacc
import concourse.tile as tile
import concourse.mybir as mybir
from concourse.bass_utils import run_bass_kernel_spmd

N_CORES = 8
U_NODES, U_DIM = 100000, 128
B_DIM = 64
OUT_DIM = 2 * U_DIM + B_DIM  # 320
P = 128
B_EDGES = 1000000
WIN = 32768  # dma_gather int16 window (rows)
N_WIN = (U_NODES + WIN - 1) // WIN  # 4
T_EDGES = 2048  # edges per supertile (= per dma_gather op)


def _round_up(x, m):
    return -(-x // m) * m


def _segment_caps(n_edges: int) -> list[int]:
    """Fixed per-(w1,w2)-combo capacities: expected + 6 sigma, to x128."""
    pw = [min(WIN, U_NODES - w * WIN) / U_NODES for w in range(N_WIN)]
    caps = []
    for w1 in range(N_WIN):
        for w2 in range(N_WIN):
            p = pw[w1] * pw[w2]
            exp = n_edges * p
            sigma = (n_edges * p * (1 - p)) ** 0.5
            caps.append(_round_up(int(exp + 6 * sigma) + 64, P))
    return caps


def _build_nc(n_edges: int, out_bufs: int = 3, u_bufs: int = 3, b_bufs: int = 3,
              t_edges: int = T_EDGES, mode: str = "gather"):
    caps = _segment_caps(n_edges)
    ne = sum(caps)
    nc = bacc.Bacc(
        "TRN2",
        target_bir_lowering=False,
        debug=False,
        enable_asserts=False,
        num_devices=N_CORES,
    )
    unary = nc.dram_tensor(
        "unary", [U_NODES, U_DIM], mybir.dt.float32, kind="ExternalInput"
    ).ap()
    binaryT = nc.dram_tensor(
        "binaryT", [P, (ne // P) * B_DIM], mybir.dt.float32, kind="ExternalInput"
    ).ap()
    idx1w = nc.dram_tensor(
        "idx1w", [P, ne // 16], mybir.dt.int16, kind="ExternalInput"
    ).ap()
    idx2w = nc.dram_tensor(
        "idx2w", [P, ne // 16], mybir.dt.int16, kind="ExternalInput"
    ).ap()
    out = nc.dram_tensor(
        "out", [ne, OUT_DIM], mybir.dt.float32, kind="ExternalOutput"
    ).ap()

    bin_v = binaryT.rearrange("p (n c) -> p n c", c=B_DIM)  # [128, ne/128, 64]
    out_v = out.rearrange("(n p) c -> p n c", p=P)  # [128, ne/128, 320]
    wins = [unary[w * WIN : min((w + 1) * WIN, U_NODES), :] for w in range(N_WIN)]

    with tile.TileContext(nc) as tc, ExitStack() as ctx:
        idx_pool = ctx.enter_context(tc.tile_pool(name="idx", bufs=1))
        ot_pool = ctx.enter_context(tc.tile_pool(name="ot", bufs=out_bufs))
        u_pool = ctx.enter_context(tc.tile_pool(name="ut", bufs=u_bufs))
        bt_pool = ctx.enter_context(tc.tile_pool(name="bt", bufs=b_bufs))

        idx1_sb = idx_pool.tile([P, ne // 16], mybir.dt.int16, tag="idx1")
        idx2_sb = idx_pool.tile([P, ne // 16], mybir.dt.int16, tag="idx2")
        nc.sync.dma_start(idx1_sb[:], idx1w[:, :])
        nc.sync.dma_start(idx2_sb[:], idx2w[:, :])

        off = 0
        for g, cap in enumerate(caps):
            w1, w2 = g // N_WIN, g % N_WIN
            a = 0
            while a < cap:
                n = min(t_edges, cap - a)
                S = n // P
                e0 = off + a  # global slot offset, multiple of 128
                ot = ot_pool.tile([P, S * OUT_DIM], mybir.dt.float32, tag="ot")
                ov = ot[:].rearrange("p (s c) -> p s c", c=OUT_DIM)
                ut1 = u_pool.tile([P, S * U_DIM], mybir.dt.float32, tag="ut1")
                ut2 = u_pool.tile([P, S * U_DIM], mybir.dt.float32, tag="ut2")
                nc.gpsimd.dma_gather(
                    ut1[:].rearrange("p (s c) -> p s c", c=U_DIM),
                    wins[w1],
                    idx1_sb[:, e0 // 16 : (e0 + n) // 16],
                    n,
                    n,
                    U_DIM,
                    single_packet=False,
                )
                nc.gpsimd.dma_gather(
                    ut2[:].rearrange("p (s c) -> p s c", c=U_DIM),
                    wins[w2],
                    idx2_sb[:, e0 // 16 : (e0 + n) // 16],
                    n,
                    n,
                    U_DIM,
                    single_packet=False,
                )
                bt = bt_pool.tile([P, S * B_DIM], mybir.dt.float32, tag="bt")
                nc.sync.dma_start(bt[:], bin_v[:, e0 // P : (e0 + n) // P, :])
                u1v = ut1[:].rearrange("p (s c) -> p s c", c=U_DIM)
                u2v = ut2[:].rearrange("p (s c) -> p s c", c=U_DIM)
                bv = bt[:].rearrange("p (s c) -> p s c", c=B_DIM)
                nc.vector.tensor_scalar_add(ov[:, :, 0:U_DIM], u1v[:, :, :], 0.0)
                nc.vector.tensor_scalar_add(
                    ov[:, :, U_DIM : 2 * U_DIM], u2v[:, :, :], 0.0
                )
                nc.scalar.copy(ov[:, :, 2 * U_DIM : OUT_DIM], bv[:, :, :])
                nc.sync.dma_start(out_v[:, e0 // P : (e0 + n) // P, :], ot[:])
                a += n
            off += cap

    nc.compile()
    return nc


_NC_CACHE: dict = {}


def _get_nc(n_edges: int):
    if n_edges not in _NC_CACHE:
        _NC_CACHE[n_edges] = _build_nc(n_edges)
    return _NC_CACHE[n_edges]


def _wrap_idx(idx_sorted: np.ndarray) -> np.ndarray:
    """[ne] int16 -> [128, ne//16] wrapped (pos i -> (i%16, i//16)), x8 replicated."""
    ne = idx_sorted.shape[0]
    w = idx_sorted.reshape(ne // 16, 16).T.astype(np.int16)  # [16, ne//16]
    return np.ascontiguousarray(np.tile(w, (8, 1)))


def _make_core_inputs(unary, b_shard, i1_shard, i2_shard, caps):
    """Sort edges by (w1,w2) combo into fixed-capacity segments."""
    ne = sum(caps)
    n = i1_shard.shape[0]
    w1 = i1_shard // WIN
    w2 = i2_shard // WIN
    combo = w1 * N_WIN + w2
    order = np.argsort(combo, kind="stable")  # shard positions in segment order
    counts = np.bincount(combo, minlength=N_WIN * N_WIN)

    i1s = np.zeros(ne, dtype=np.int32)
    i2s = np.zeros(ne, dtype=np.int32)
    bs = np.zeros((ne, B_DIM), dtype=np.float32)
    valid_rows = np.empty(n, dtype=np.int64)  # padded-slot row for each sorted edge
    off = 0
    src = 0
    for g, cap in enumerate(caps):
        c = counts[g]
        if c > cap:
            raise ValueError(f"segment {g} overflow: {c} > {cap}")
        sel = order[src : src + c]
        i1s[off : off + c] = i1_shard[sel] - (g // N_WIN) * WIN
        i2s[off : off + c] = i2_shard[sel] - (g % N_WIN) * WIN
        bs[off : off + c] = b_shard[sel]
        valid_rows[src : src + c] = off + np.arange(c)
        # padding rows keep idx 0 (valid in every window)
        off += cap
        src += c

    binT = np.ascontiguousarray(
        bs.reshape(ne // P, P, B_DIM).transpose(1, 0, 2).reshape(P, -1)
    )
    in_map = {
        "unary": unary,
        "binaryT": binT,
        "idx1w": _wrap_idx(i1s),
        "idx2w": _wrap_idx(i2s),
    }
    return in_map, order, valid_rows


def kernel(unary, binary, index1, index2):
    unary = np.ascontiguousarray(np.asarray(unary, dtype=np.float32))
    binary = np.ascontiguousarray(np.asarray(binary, dtype=np.float32))
    index1 = np.asarray(index1).astype(np.int32).ravel()
    index2 = np.asarray(index2).astype(np.int32).ravel()

    ne_total = binary.shape[0]
    per_core = -(-ne_total // N_CORES)
    caps = _segment_caps(per_core)
    nc = _get_nc(per_core)

    in_maps, orders, valids, bounds = [], [], [], []
    for c in range(N_CORES):
        lo = c * per_core
        hi = min(lo + per_core, ne_total)
        in_map, order, valid_rows = _make_core_inputs(
            unary, binary[lo:hi], index1[lo:hi], index2[lo:hi], caps
        )
        in_maps.append(in_map)
        orders.append(order)
        valids.append(valid_rows)
        bounds.append((lo, hi))

    out = np.empty((ne_total, OUT_DIM), dtype=np.float32)
    for attempt in range(3):
        res = run_bass_kernel_spmd(nc, in_maps, core_ids=list(range(N_CORES)))
        for c in range(N_CORES):
            lo, hi = bounds[c]
            out[lo + orders[c]] = res.results[c]["out"][valids[c]]
        # Transient device faults (e.g. a wedged SDMA ring from a prior
        # crashed session) can corrupt rows nondeterministically; verify
        # against the inputs and retry on a fresh session if needed.
        sample = np.random.default_rng(attempt).integers(0, ne_total, 65536)
        ok = (
            np.array_equal(out[sample, :U_DIM], unary[index1[sample]])
            and np.array_equal(out[sample, U_DIM : 2 * U_DIM], unary[index2[sample]])
            and np.array_equal(out[sample, 2 * U_DIM :], binary[sample])
        )
        if ok:
            break
    return out
